# revision 17
# baseline (speedup 1.0000x reference)
"""Trainium2 Bass kernel for nn_CrossAttention (b=4, lq=lkv=2048, dq=1024, dkv=768, 4 heads).

Sharding: 8 cores = (batch b in 0..3) x (head-group g in 0..1); each core handles
one batch and 2 of the 4 heads (512 of the 1024 head dims).  All activations are
fed to the device pre-transposed ([model_dim, seq]) so every matmul contracts
over the partition dimension with zero on-device transposes:

  qhT  [512,2048] = WqT.T @ qT          (proj, contraction over dq=1024)
  khT  [512,2048] = WkT.T @ kvT         (proj, contraction over dkv=768)
  vh   [2048,512] = kvT_chunk.T @ WvT   (proj, natural layout)
  sT   [2048,2048] per head = khT_h.T @ qhT_h    (scoresT: lkv on partitions)
  eT   = exp(sT / 16 - 4)               (shift instead of max-subtraction keeps
                                         eT and its fp16 tree sums in fp16 range;
                                         the shift cancels in softmax)
  ctxT [256,2048] per head accumulated over lkv tiles (lhsT=vh, rhs=eT)
  sum  via DVE fp16 add-tree over eT tiles + one ones[128,128] matmul
        (every psum partition gets the column sum -> reciprocal)
  ctxT normalized by DVE mul with the reciprocal tile; the normalization
        tail for chunk i is emitted inside chunk i+1 so PE never stalls
  outT [1024,2048] = WoT.T @ ctxT       (output proj over the core's 512 dims)

All matmul operands are fp16: full-rate (1 cyc/row) like f32r, but LDWEIGHTS
is 2-byte (2x faster stationary loads -> hides under the 512-row matmuls),
DMA traffic halves, and the softmax sum-tree runs at 2x on DVE.  Measured
accuracy ~2e-3 max-rel (threshold 2e-2).
Host gathers: out[b] = (outT[core 2b] + outT[core 2b+1]).T + bo.
"""

import numpy as np

B = 4
LQ = 2048
LKV = 2048
DQ = 1024
DKV = 768
HD = 256  # per-head dim
GH = 512  # head dims per core (2 heads)
P = 128
NCORES = 8
NQ = LQ // 512  # lq chunks of 512
KT_Q = DQ // P  # 8
KT_KV = DKV // P  # 6
KT_L = LKV // P  # 16

ESHIFT = -4.0  # exp(s/16 + ESHIFT); cancels in softmax, keeps fp16 in range
TRACE = False

_COMPILED = None
last_exec_time_ns = None
last_profile = None


def _emit(tc, aps):
    from contextlib import ExitStack

    import concourse.mybir as mybir

    nc = tc.nc
    f32 = mybir.dt.float32
    dt16 = mybir.dt.float16
    Exp = mybir.ActivationFunctionType.Exp

    qT, kvT, WqT, WkT, WvT, WoT, outT = (
        aps["qT"], aps["kvT"], aps["WqT"], aps["WkT"], aps["WvT"], aps["WoT"],
        aps["outT"],
    )
    kvT_r = kvT.rearrange("(k p) n -> p k n", p=P)  # [128, 6, 2048]
    qT_r = qT.rearrange("(k p) n -> p k n", p=P)    # [128, 8, 2048]
    WkT_r = WkT.rearrange("(k p) g -> p k g", p=P)  # [128, 6, 512]
    WvT_r = WvT.rearrange("(k p) g -> p k g", p=P)
    WqT_r = WqT.rearrange("(k p) g -> p k g", p=P)  # [128, 8, 512]
    WoT_r = WoT.rearrange("(k p) d -> p k d", p=P)  # [128, 4, 1024]

    with ExitStack() as top:
        # persistent SBUF tensors
        khT_pool = top.enter_context(tc.tile_pool(name="khT", bufs=1))
        qhT_pool = top.enter_context(tc.tile_pool(name="qhT", bufs=1))
        vh_pool = top.enter_context(tc.tile_pool(name="vh", bufs=1))
        const_pool = top.enter_context(tc.tile_pool(name="const", bufs=1))

        khT = [khT_pool.tile([P, LKV], dt16, tag=f"khT{i}", name=f"khT{i}")
               for i in range(4)]
        qhT = [qhT_pool.tile([P, LQ], dt16, tag=f"qhT{i}", name=f"qhT{i}")
               for i in range(4)]
        vh = [vh_pool.tile([P, GH], dt16, tag=f"vh{i}", name=f"vh{i}")
              for i in range(KT_L)]

        ones_sq = const_pool.tile([P, P], dt16, tag="ones_sq", name="ones_sq")
        ones_f32 = const_pool.tile([P, P], f32, tag="ones_f32", name="ones_f32")
        nc.vector.memset(ones_f32[:], 1.0)
        nc.vector.tensor_copy(ones_sq[:], ones_f32[:])
        ebias = const_pool.tile([P, 1], f32, tag="ebias", name="ebias")
        nc.vector.memset(ebias[:], ESHIFT)

        # ---------------- Phase A: projections ----------------
        with ExitStack() as ph:
            w_pool = ph.enter_context(tc.tile_pool(name="w", bufs=1))
            kvc_pool = ph.enter_context(tc.tile_pool(name="kvc", bufs=4))
            qc_pool = ph.enter_context(tc.tile_pool(name="qc", bufs=2))
            psA = ph.enter_context(tc.tile_pool(name="psA", bufs=4, space="PSUM"))

            wk_t = w_pool.tile([P, KT_KV, GH], dt16, tag="wk", name="wk")
            wv_t = w_pool.tile([P, KT_KV, GH], dt16, tag="wv", name="wv")
            wq_t = w_pool.tile([P, KT_Q, GH], dt16, tag="wq", name="wq")
            kvc0 = kvc_pool.tile([P, KT_KV, 512], dt16, tag="kvc", name="kvc")
            for kt in range(KT_KV):  # interleaved per-kt so mm kt0 starts early
                nc.sync.dma_start(wv_t[:, kt, :], WvT_r[:, kt, :])
                nc.sync.dma_start(kvc0[:, kt, :], kvT_r[:, kt, 0:512])

            kvc_tiles = {0: kvc0}
            qc_tiles = {}

            def load_kvc(n):
                if n in kvc_tiles or n >= NQ:
                    return
                t = kvc_pool.tile([P, KT_KV, 512], dt16, tag="kvc", name="kvc")
                nc.sync.dma_start(t[:], kvT_r[:, :, n * 512:(n + 1) * 512])
                kvc_tiles[n] = t

            def load_qc(n):
                if n in qc_tiles or n >= NQ:
                    return
                t = qc_pool.tile([P, KT_Q, 512], dt16, tag="qc", name="qc")
                nc.sync.dma_start(t[:], qT_r[:, :, n * 512:(n + 1) * 512])
                qc_tiles[n] = t

            def emit_vh(n):
                # kt-outer so the first matmuls only need the first kt-slice
                # of the chunk (lets compute start while the DMA streams in)
                kvc = kvc_tiles[n]
                ps = [psA.tile([P, 512], f32, tag="psA", name="psA")
                      for _ in range(4)]
                for kt in range(KT_KV):
                    for lj in range(4):
                        nc.tensor.matmul(
                            ps[lj][:],
                            lhsT=kvc[:, kt, lj * P:(lj + 1) * P],
                            rhs=wv_t[:, kt, :],
                            start=(kt == 0),
                            stop=(kt == KT_KV - 1),
                        )
                for lj in range(4):
                    nc.vector.tensor_copy(vh[4 * n + lj][:], ps[lj][:])

            def emit_khT(np_):
                # one stationary load feeds both 512-chunks of the pair
                n0, n1 = 2 * np_, 2 * np_ + 1
                for m in range(4):
                    ps0 = psA.tile([P, 512], f32, tag="psA", name="psA")
                    ps1 = psA.tile([P, 512], f32, tag="psA", name="psA")
                    for kt in range(KT_KV):
                        nc.tensor.matmul(
                            ps0[:],
                            lhsT=wk_t[:, kt, m * P:(m + 1) * P],
                            rhs=kvc_tiles[n0][:, kt, :],
                            start=(kt == 0),
                            stop=(kt == KT_KV - 1),
                        )
                        nc.tensor.matmul(
                            ps1[:],
                            lhsT=wk_t[:, kt, m * P:(m + 1) * P],
                            rhs=kvc_tiles[n1][:, kt, :],
                            start=(kt == 0),
                            stop=(kt == KT_KV - 1),
                        )
                    nc.vector.tensor_copy(khT[m][:, n0 * 512:(n0 + 1) * 512],
                                          ps0[:])
                    nc.vector.tensor_copy(khT[m][:, n1 * 512:(n1 + 1) * 512],
                                          ps1[:])

            def emit_qhT(n):
                nsl = slice(n * 512, (n + 1) * 512)
                qc = qc_tiles[n]
                for m in range(4):  # qhT head-dim tiles
                    ps = psA.tile([P, 512], f32, tag="psA", name="psA")
                    for kt in range(KT_Q):
                        nc.tensor.matmul(
                            ps[:],
                            lhsT=wq_t[:, kt, m * P:(m + 1) * P],
                            rhs=qc[:, kt, :],
                            start=(kt == 0),
                            stop=(kt == KT_Q - 1),
                        )
                    nc.vector.tensor_copy(qhT[m][:, nsl], ps[:])

            # chunk-paced schedule: vh leads (needs only its own chunk), khT
            # consumes chunk pairs, qhT trails.  DMA order matches first use.
            load_kvc(1)
            nc.sync.dma_start(wk_t[:], WkT_r[:])
            load_qc(0)
            load_kvc(2)
            nc.sync.dma_start(wq_t[:], WqT_r[:])
            load_kvc(3)
            load_qc(1)
            emit_vh(0)
            emit_vh(1)
            emit_khT(0)
            load_qc(2)
            emit_vh(2)
            emit_qhT(0)
            load_qc(3)
            emit_vh(3)
            emit_khT(1)
            emit_qhT(1)
            emit_qhT(2)
            emit_qhT(3)

        # ---------------- Phases B+C ----------------
        bc_top = top.enter_context(ExitStack())
        ctxT_pool = bc_top.enter_context(tc.tile_pool(name="ctxT", bufs=1))
        ctxT = [ctxT_pool.tile([P, LQ], dt16, tag=f"ctxT{i}", name=f"ctxT{i}")
                for i in range(4)]

        wo_pool = bc_top.enter_context(tc.tile_pool(name="wo", bufs=1))
        wo_t = wo_pool.tile([P, 4, DQ], dt16, tag="wo", name="wo")
        nc.sync.dma_start(wo_t[:], WoT_r[:])
        ps_sum = bc_top.enter_context(tc.tile_pool(name="ps_sum", bufs=2,
                                                   space="PSUM"))
        ps_ctx = bc_top.enter_context(tc.tile_pool(name="ps_ctx", bufs=4,
                                                   space="PSUM"))
        acc_pool = bc_top.enter_context(tc.tile_pool(name="acc", bufs=2))
        rcb_pool = bc_top.enter_context(tc.tile_pool(name="rcb", bufs=2))

        # ---------------- Phase B: attention per head ----------------
        with ExitStack() as ph:
            ps_s = ph.enter_context(tc.tile_pool(name="ps_s", bufs=2, space="PSUM"))
            et_pool = ph.enter_context(tc.tile_pool(name="et", bufs=6))
            g_pool = ph.enter_context(tc.tile_pool(name="g", bufs=2))

            scale = 1.0 / np.sqrt(HD)
            pending_tail = [None, None]  # [pss+recip, muls]

            def flush_tail1():
                if pending_tail[0] is not None:
                    pending_tail[0]()
                    pending_tail[0] = None

            def flush_tail():
                flush_tail1()
                if pending_tail[1] is not None:
                    pending_tail[1]()
                    pending_tail[1] = None

            for h in range(2):
                k0, k1 = khT[2 * h], khT[2 * h + 1]
                q0, q1 = qhT[2 * h], qhT[2 * h + 1]
                hsl0 = slice(HD * h, HD * h + P)
                hsl1 = slice(HD * h + P, HD * h + 2 * P)
                for n in range(NQ):
                    nsl = slice(n * 512, (n + 1) * 512)
                    pc0 = ps_ctx.tile([P, 512], f32, tag="pc", name="pc")
                    pc1 = ps_ctx.tile([P, 512], f32, tag="pc", name="pc")
                    g = [None] * 4

                    et_prev = None
                    for kt in range(KT_L):
                        ksl = slice(kt * P, (kt + 1) * P)
                        ps = ps_s.tile([P, 512], f32, tag="ps_s", name="ps_s")
                        nc.tensor.matmul(
                            ps[:], lhsT=k0[:, ksl], rhs=q0[:, nsl],
                            start=True, stop=False,
                        )
                        nc.tensor.matmul(
                            ps[:], lhsT=k1[:, ksl], rhs=q1[:, nsl],
                            start=False, stop=True,
                        )
                        et = et_pool.tile([P, 512], dt16, tag="et", name="et")
                        nc.scalar.activation(et[:], ps[:], Exp, scale=scale,
                                             bias=ebias[:])

                        # sumexp tree accumulation on DVE (all-fp16: 2x rate)
                        j = kt // 4
                        if kt % 4 == 0:
                            g[j] = g_pool.tile([P, 512], dt16, tag=f"g{j}",
                                               name=f"g{j}")
                            nc.vector.tensor_copy(g[j][:], et[:])
                        else:
                            nc.vector.tensor_add(g[j][:], g[j][:], et[:])

                        if kt == 1:
                            flush_tail1()
                        elif kt == 2:
                            flush_tail()

                        if et_prev is not None:
                            pkt, pet = et_prev
                            nc.tensor.matmul(
                                pc0[:], lhsT=vh[pkt][:, hsl0], rhs=pet[:],
                                start=(pkt == 0), stop=False,
                            )
                            nc.tensor.matmul(
                                pc1[:], lhsT=vh[pkt][:, hsl1], rhs=pet[:],
                                start=(pkt == 0), stop=False,
                            )
                        et_prev = (kt, et)

                    pkt, pet = et_prev
                    nc.tensor.matmul(pc0[:], lhsT=vh[pkt][:, hsl0], rhs=pet[:],
                                     start=False, stop=True)
                    nc.tensor.matmul(pc1[:], lhsT=vh[pkt][:, hsl1], rhs=pet[:],
                                     start=False, stop=True)

                    # finish the tree: acc = (g0+g1) + (g2+g3), fp16 throughout
                    g01 = g_pool.tile([P, 512], dt16, tag="g01", name="g01")
                    nc.vector.tensor_add(g01[:], g[0][:], g[1][:])
                    g23 = g_pool.tile([P, 512], dt16, tag="g23", name="g23")
                    nc.vector.tensor_add(g23[:], g[2][:], g[3][:])
                    acc = acc_pool.tile([P, 512], dt16, tag="acc", name="acc")
                    nc.vector.tensor_add(acc[:], g01[:], g23[:])

                    def make_tails(pc0=pc0, pc1=pc1, acc=acc, h=h, nsl=nsl):
                        cell = {}

                        def tail1():
                            pss = ps_sum.tile([P, 512], f32, tag="pss",
                                              name="pss")
                            nc.tensor.matmul(pss[:], lhsT=ones_sq[:],
                                             rhs=acc[:], start=True, stop=True)
                            rcb = rcb_pool.tile([P, 512], f32, tag="rcb",
                                                name="rcb")
                            nc.vector.reciprocal(rcb[:], pss[:])
                            cell["rcb"] = rcb

                        def tail2():
                            rcb = cell["rcb"]
                            nc.vector.tensor_mul(ctxT[2 * h][:, nsl], pc0[:],
                                                 rcb[:])
                            nc.vector.tensor_mul(ctxT[2 * h + 1][:, nsl],
                                                 pc1[:], rcb[:])
                        return tail1, tail2

                    pending_tail[0], pending_tail[1] = make_tails()

        # ---------------- Phase C: output projection ----------------
        # n-outer: the last B chunk's normalization (ctxT cols n=3) is flushed
        # up front and has the n=0..2 columns (~20us of matmuls) as runway
        with ExitStack() as ph:
            psC = ph.enter_context(tc.tile_pool(name="psC", bufs=2, space="PSUM"))
            outC = ph.enter_context(tc.tile_pool(name="outC", bufs=4))

            flush_tail()
            for n in range(NQ):  # 4
                nsl = slice(n * 512, (n + 1) * 512)
                for m in range(DQ // P):  # 8
                    ps = psC.tile([P, 512], f32, tag="psC", name="psC")
                    for kt in range(4):
                        nc.tensor.matmul(
                            ps[:],
                            lhsT=wo_t[:, kt, m * P:(m + 1) * P],
                            rhs=ctxT[kt][:, nsl],
                            start=(kt == 0),
                            stop=(kt == 3),
                        )
                    ot = outC.tile([P, 512], dt16, tag="ot", name="ot")
                    nc.scalar.copy(ot[:], ps[:])
                    nc.sync.dma_start(outT[m * P:(m + 1) * P, nsl], ot[:])


def _build():
    import concourse.bacc as bacc
    import concourse.mybir as mybir
    import concourse.tile as tile

    dt16 = mybir.dt.float16
    nc = bacc.Bacc("TRN2", target_bir_lowering=False, debug=False)
    aps = {
        "qT": nc.dram_tensor("qT", [DQ, LQ], dt16, kind="ExternalInput").ap(),
        "kvT": nc.dram_tensor("kvT", [DKV, LKV], dt16, kind="ExternalInput").ap(),
        "WqT": nc.dram_tensor("WqT", [DQ, GH], dt16, kind="ExternalInput").ap(),
        "WkT": nc.dram_tensor("WkT", [DKV, GH], dt16, kind="ExternalInput").ap(),
        "WvT": nc.dram_tensor("WvT", [DKV, GH], dt16, kind="ExternalInput").ap(),
        "WoT": nc.dram_tensor("WoT", [GH, DQ], dt16, kind="ExternalInput").ap(),
        "outT": nc.dram_tensor("outT", [DQ, LQ], dt16, kind="ExternalOutput").ap(),
    }
    with tile.TileContext(nc) as tc:
        _emit(tc, aps)
    nc.compile()
    return nc


def make_in_maps(q, kv, Wq, Wk, Wv, Wo):
    in_maps = []
    for c in range(NCORES):
        b, g = divmod(c, 2)
        hs = slice(g * GH, (g + 1) * GH)
        in_maps.append({
            "qT": np.ascontiguousarray(q[b].T, dtype=np.float16),
            "kvT": np.ascontiguousarray(kv[b].T, dtype=np.float16),
            "WqT": np.ascontiguousarray(Wq[hs, :].T, dtype=np.float16),
            "WkT": np.ascontiguousarray(Wk[hs, :].T, dtype=np.float16),
            "WvT": np.ascontiguousarray(Wv[hs, :].T, dtype=np.float16),
            "WoT": np.ascontiguousarray(Wo[:, hs].T, dtype=np.float16),
        })
    return in_maps


def kernel(q, kv, Wq, Wk, Wv, Wo, bo):
    global _COMPILED, last_exec_time_ns, last_profile
    from concourse.bass_utils import run_bass_kernel_spmd

    if _COMPILED is None:
        _COMPILED = _build()
    nc = _COMPILED

    q = np.asarray(q, np.float32)
    kv = np.asarray(kv, np.float32)
    Wq = np.asarray(Wq, np.float32)
    Wk = np.asarray(Wk, np.float32)
    Wv = np.asarray(Wv, np.float32)
    Wo = np.asarray(Wo, np.float32)
    bo = np.asarray(bo, np.float32)

    in_maps = make_in_maps(q, kv, Wq, Wk, Wv, Wo)
    res = run_bass_kernel_spmd(nc, in_maps, core_ids=list(range(NCORES)),
                               trace=TRACE)
    last_exec_time_ns = res.exec_time_ns
    last_profile = res.profile_json

    out = np.empty((B, LQ, DQ), np.float32)
    for b in range(B):
        acc = (res.results[2 * b]["outT"].astype(np.float32)
               + res.results[2 * b + 1]["outT"].astype(np.float32))
        out[b] = acc.T + bo
    return out
